# revision 19
# baseline (speedup 1.0000x reference)
"""Trainium2 Bass kernel for nn_AttentionOnDetail (dense transformer block with
low-rank qkv projection, rope, sigmoid-gated causal linear attention, low-rank
gated output projection).

Sharding: 8 cores = (batch b in 0..3) x (head-group hg in 0..1).  Each core
computes its 8 heads over the full sequence, produces a partial cproj_w1
projection p [256, T]; a per-pair ReduceScatter(add) sums the two head-group
partials and hands each core its token-half, on which it runs cproj_w2, the
silu gating and the residual.  Output: each core returns its [T/2, C] slice.

On-core layout: activations flow in [feature, token] layout so every matmul
contraction runs over the partition axis.  The 2048 qkv feature rows are
permuted (host-side weight reordering) so rope-active row blocks are packed
into full 128-partition tiles:
  per 4-head group g, tiles A=[dh 0:32 x4 heads], B=[dh 64:96], C=[dh 32:64],
  D=[dh 96:128]; rope touches only A,B: outA = A*c4 + B*s4, outB = B*c4 - A*s4.
rmsnorm of q/k is computed via selector-matrix matmuls (column sums of
squares on the tensor engine), applied through broadcast-matmul row tiles.
The causal cumsum is the hardware prefix scan (tensor_tensor_scan) along the
free (token) axis, chained across token slices via a carried state column.
"""
import sys
import numpy as np

for _p in ("/opt/trn_rl_repo", "/opt/pypackages"):
    if _p not in sys.path:
        sys.path.insert(0, _p)

import concourse.bass as bass
import concourse.bacc as bacc
import concourse.mybir as mybir
import concourse.tile as tile
from concourse.masks import make_identity
from concourse import bass_utils

F32 = mybir.dt.float32
F32R = mybir.dt.float32r
BF16 = mybir.dt.bfloat16
AF = mybir.ActivationFunctionType
OP = mybir.AluOpType

B, T_FULL, C = 4, 4096, 1024
NH, DH = 16, 128
DQKV, DR = 2048, 256
EPS = float(np.finfo(np.float32).eps)
NCORES = 8
HPG = 4                # heads per packed group
NGRP = 2               # groups per core (8 heads/core)
TS_DEFAULT = 512           # token slice


# --------------------------------------------------------------------------
# host-side packing helpers
# --------------------------------------------------------------------------

def _packed_rows(hg):
    """8 tiles x 128 global dqkv row indices, order g0 A,B,C,D then g1 A,B,C,D."""
    tiles = []
    for g in range(NGRP):
        for dh0 in (0, 64, 32, 96):       # A, B, C, D
            rows = []
            for i in range(HPG):
                h = hg * 8 + g * HPG + i
                rows.extend(h * 128 + dh0 + j for j in range(32))
            tiles.append(rows)
    return tiles


def _tables(t_len):
    quarter = DH // 4
    freq = (1.0 / 1024.0) ** np.linspace(0.0, 1.0, quarter).astype(np.float32)
    pos = np.arange(t_len, dtype=np.float32)
    theta = pos[None, :] * freq[:, None]
    c4 = np.tile(np.cos(theta), (4, 1)).astype(np.float32)   # [128, T]
    s4 = np.tile(np.sin(theta), (4, 1)).astype(np.float32)
    return c4, s4


def _sel4():
    m = np.zeros((128, 4), np.float32)
    for k in range(128):
        m[k, k // 32] = 1.0
    return m


def _core_arrays(x_b, w1, w2, cw1, cw2, hg, t_len):
    tiles = _packed_rows(hg)
    lhsT = []
    for base in (0, DQKV, 2 * DQKV):
        for rows in tiles:
            lhsT.append(w2[np.array(rows) + base, :].T)
    qkv_lhsT = np.ascontiguousarray(np.stack(lhsT))          # [24, 256, 128]
    w1t = np.ascontiguousarray(w1.T)                          # [1024, 256]
    local_cols = np.concatenate([np.array(r) for r in tiles])
    w1c_T = np.ascontiguousarray(cw1[:, local_cols].T)        # [1024, 256]
    w2c_T = np.ascontiguousarray(cw2.T)                       # [256, 2048]
    th = t_len // 2
    xres = np.ascontiguousarray(x_b[hg * th:(hg + 1) * th, :])
    import ml_dtypes
    bf = lambda a: np.ascontiguousarray(a.astype(ml_dtypes.bfloat16))
    return dict(x=np.ascontiguousarray(x_b), xres=xres, wq=bf(qkv_lhsT),
                w1t=bf(w1t), w1c=bf(w1c_T), w2c=bf(w2c_T))


# --------------------------------------------------------------------------
# device kernel
# --------------------------------------------------------------------------

def _build(t_len, TS):
    ns = t_len // TS          # number of token slices
    th = t_len // 2           # this core's output token-half
    ns2 = th // TS            # output slices

    nc = bacc.Bacc("TRN2", target_bir_lowering=False, debug=False,
                   num_devices=NCORES)

    x_d = nc.dram_tensor("x", [t_len, C], F32, kind="ExternalInput").ap()
    xres_d = nc.dram_tensor("xres", [th, C], F32, kind="ExternalInput").ap()
    wq_d = nc.dram_tensor("wq", [24, 256, 128], BF16, kind="ExternalInput").ap()
    w1t_d = nc.dram_tensor("w1t", [C, DR], BF16, kind="ExternalInput").ap()
    w1c_d = nc.dram_tensor("w1c", [1024, DR], BF16, kind="ExternalInput").ap()
    w2c_d = nc.dram_tensor("w2c", [DR, 2 * C], BF16, kind="ExternalInput").ap()
    c4_d = nc.dram_tensor("c4", [128, t_len], F32, kind="ExternalInput").ap()
    s4_d = nc.dram_tensor("s4", [128, t_len], F32, kind="ExternalInput").ap()
    sel4_d = nc.dram_tensor("sel4", [128, 4], BF16, kind="ExternalInput").ap()
    bs4_d = nc.dram_tensor("bs4", [4, 128], BF16, kind="ExternalInput").ap()
    out_d = nc.dram_tensor("out", [th, C], F32, kind="ExternalOutput").ap()

    with tile.TileContext(nc) as tc:
        _body(tc, TS, ns, ns2, th, x_d, xres_d, wq_d, w1t_d, w1c_d, w2c_d,
              c4_d, s4_d, sel4_d, bs4_d, out_d)
    return nc


def _body(tc, TS, ns, ns2, th, x_d, xres_d, wq_d, w1t_d, w1c_d, w2c_d,
          c4_d, s4_d, sel4_d, bs4_d, out_d):
    nc = tc.nc
    from contextlib import ExitStack
    ctx = ExitStack()
    with ctx:
        # ---- pools ----
        wpool = ctx.enter_context(tc.tile_pool(name="weights", bufs=1))
        xpool = ctx.enter_context(tc.tile_pool(name="x", bufs=4))
        xnpool = ctx.enter_context(tc.tile_pool(name="xn", bufs=2))
        xntp = ctx.enter_context(tc.tile_pool(name="xnt", bufs=2))
        hpool = ctx.enter_context(tc.tile_pool(name="h", bufs=2))
        scr = ctx.enter_context(tc.tile_pool(name="scr", bufs=3))
        xsqp = ctx.enter_context(tc.tile_pool(name="xsq", bufs=1))
        ropep = ctx.enter_context(tc.tile_pool(name="rope", bufs=3))
        qkp = ctx.enter_context(tc.tile_pool(name="qk", bufs=18))
        rqp = ctx.enter_context(tc.tile_pool(name="rq", bufs=10))
        kvp = ctx.enter_context(tc.tile_pool(name="kv", bufs=10))
        srow = ctx.enter_context(tc.tile_pool(name="srow", bufs=2))
        small = ctx.enter_context(tc.tile_pool(name="small", bufs=8))
        statep = ctx.enter_context(tc.tile_pool(name="state", bufs=1))
        tabp = ctx.enter_context(tc.tile_pool(name="tab", bufs=2))
        outp = ctx.enter_context(tc.tile_pool(name="outp", bufs=1))
        usgp = ctx.enter_context(tc.tile_pool(name="usg", bufs=2))
        pbp = ctx.enter_context(tc.tile_pool(name="pb", bufs=2))

        mmps = ctx.enter_context(tc.tile_pool(name="mmps", bufs=4, space="PSUM"))
        ppps = ctx.enter_context(tc.tile_pool(name="ppps", bufs=1, space="PSUM"))
        ssqps = ctx.enter_context(tc.tile_pool(name="ssqps", bufs=2, space="PSUM"))

        dram = ctx.enter_context(tc.tile_pool(name="dram", bufs=1, space="DRAM"))

        # ---- static weights ----
        wq_sb = wpool.tile([128, 24 * 2 * 128], BF16)
        for m in range(24):
            for kt in range(2):
                blk = m * 2 + kt
                nc.sync.dma_start(wq_sb[:, blk * 128:(blk + 1) * 128],
                                  wq_d[m, kt * 128:(kt + 1) * 128, :])
        w1t_sb = wpool.tile([128, 8 * 256], BF16)
        w1c_sb = wpool.tile([128, 8 * 256], BF16)
        for kt in range(8):
            nc.sync.dma_start(w1t_sb[:, kt * 256:(kt + 1) * 256],
                              w1t_d[kt * 128:(kt + 1) * 128, :])
            nc.sync.dma_start(w1c_sb[:, kt * 256:(kt + 1) * 256],
                              w1c_d[kt * 128:(kt + 1) * 128, :])
        w2c_sb = wpool.tile([128, 2 * 2048], BF16)
        for kt in range(2):
            nc.sync.dma_start(w2c_sb[:, kt * 2048:(kt + 1) * 2048],
                              w2c_d[kt * 128:(kt + 1) * 128, :])
        sel4_sb = wpool.tile([128, 4], BF16)
        nc.sync.dma_start(sel4_sb[:], sel4_d)
        bs4_sb = wpool.tile([4, 128], BF16)
        nc.sync.dma_start(bs4_sb[:], bs4_d)
        ident = wpool.tile([128, 128], F32)
        make_identity(nc, ident)
        eps_sb = wpool.tile([128, 1], F32)
        nc.vector.memset(eps_sb[:], EPS)

        state = statep.tile([128, 8], F32)
        nc.vector.memset(state[:], 0.0)

        p_in = dram.tile([2, 2, 128, th], BF16)
        p_out = dram.tile([2, 128, th], BF16)

        def mm(out, lhsT, rhs, start, stop):
            nc.tensor.matmul(out, lhsT, rhs, start=start, stop=stop)

        # q/k/v lhsT slice for packed M-tile m of tensor `tens`, K-tile kt
        def wq_ap(tens, m, kt):
            base = (tens * 8 + m) * 2 + kt
            return wq_sb[:, base * 128:(base + 1) * 128]

        # ================= main slice loop =================
        for s in range(ns):
            t0 = s * TS
            # ---- tables for this slice ----
            c4_sb = tabp.tile([128, TS], F32, tag="c4")
            nc.sync.dma_start(c4_sb[:], c4_d[:, t0:t0 + TS])
            s4_sb = tabp.tile([128, TS], F32, tag="s4")
            nc.sync.dma_start(s4_sb[:], s4_d[:, t0:t0 + TS])

            # ---- x-prep: rmsnorm + transpose ----
            xnT = xntp.tile([128, 8 * TS], BF16, tag="xnT")
            for tt in range(TS // 128):
                xt = xpool.tile([128, C], F32, tag="x")
                nc.sync.dma_start(xt[:], x_d[t0 + tt * 128:t0 + (tt + 1) * 128, :])
                xsq = xsqp.tile([128, C], F32, tag="xsq")
                ssqc = small.tile([128, 1], F32, tag="ssqc")
                nc.scalar.activation(xsq[:], xt[:], AF.Square, accum_out=ssqc[:])
                rms = small.tile([128, 1], F32, tag="rms")
                nc.scalar.activation(rms[:], ssqc[:], AF.Sqrt,
                                     bias=eps_sb[:, 0:1], scale=1.0 / C)
                rstd = small.tile([128, 1], F32, tag="rstd")
                nc.vector.reciprocal(rstd[:], rms[:])
                xn = xnpool.tile([128, C], F32, tag="xn")
                nc.vector.tensor_scalar_mul(xn[:], xt[:], rstd[:, 0:1])
                for ch in range(2):
                    tp = mmps.tile([128, 512], F32, tag="mm", name="tp")
                    for cc in range(4):
                        nc.tensor.transpose(
                            tp[:, cc * 128:(cc + 1) * 128],
                            xn[:, (ch * 4 + cc) * 128:(ch * 4 + cc + 1) * 128],
                            ident[:])
                    # one strided evacuation per 4 transposed blocks
                    nc.vector.tensor_copy(
                        xnT.rearrange("p (c t) -> p c t", c=8)
                           [:, ch * 4:(ch + 1) * 4, tt * 128:(tt + 1) * 128],
                        tp.rearrange("p (c t) -> p c t", c=4))

            # ---- h = relu(rmsnorm(x) @ w1.T)^2, in [DR, TS] layout ----
            hT = hpool.tile([128, 2 * TS], BF16, tag="hT")
            for mt in range(2):
                hp = mmps.tile([128, TS], F32, tag="mm")
                for kt in range(8):
                    mm(hp[:], w1t_sb[:, kt * 256 + mt * 128:kt * 256 + (mt + 1) * 128],
                       xnT[:, kt * TS:(kt + 1) * TS], kt == 0, kt == 7)
                hc = scr.tile([128, TS], F32, tag="hc")
                nc.scalar.copy(hc[:], hp[:])
                nc.vector.scalar_tensor_tensor(
                    hT[:, mt * TS:(mt + 1) * TS], hc[:], 0.0, hc[:],
                    op0=OP.max, op1=OP.mult)

            # ---- attention: qk matmuls + squares + colsums + rope ----
            # (ACT uses only Square/Copy here -- both live in every table set)
            pp = [ppps.tile([128, TS], F32, tag=f"pp{rt}", name=f"pp{rt}")
                  for rt in range(2)]
            qk_sb = {}
            ssq_ps = {}
            for g in range(2):
                ssq_t = ssqps.tile([36, TS], F32, tag="ssq", name=f"ssq{g}")
                ssq_ps[g] = ssq_t
                for tens in range(2):          # 0 = q, 1 = k
                    ropein = {}
                    for ti in range(4):
                        qp = mmps.tile([128, TS], F32, tag="mm")
                        for kt in range(2):
                            mm(qp[:], wq_ap(tens, g * 4 + ti, kt),
                               hT[:, kt * TS:(kt + 1) * TS], kt == 0, kt == 1)
                        sqt = scr.tile([128, TS], BF16, tag="sqt")
                        nc.scalar.square(sqt[:], qp[:])
                        nc.tensor.matmul(ssq_t[tens * 32:tens * 32 + 4, :],
                                         sel4_sb[:], sqt[:],
                                         start=(ti == 0), stop=(ti == 3))
                        if ti < 2:
                            ropein[ti] = qp
                        else:
                            cd = qkp.tile([128, TS], BF16, tag="qk", name="cd")
                            nc.scalar.copy(cd[:], qp[:])
                            qk_sb[(tens, g, ti)] = cd
                    # rope: A = z1*c + z2*s ; B = z2*c - z1*s
                    ta = qkp.tile([128, TS], BF16, tag="qk", name="ta")
                    nc.vector.tensor_tensor(ta[:], ropein[0][:], c4_sb[:], OP.mult)
                    t2 = ropep.tile([128, TS], BF16, tag="t2")
                    nc.vector.tensor_tensor(t2[:], ropein[1][:], s4_sb[:], OP.mult)
                    nc.gpsimd.tensor_tensor(ta[:], ta[:], t2[:], OP.add)
                    tb = qkp.tile([128, TS], BF16, tag="qk", name="tb")
                    nc.vector.tensor_tensor(tb[:], ropein[1][:], c4_sb[:], OP.mult)
                    t4 = ropep.tile([128, TS], BF16, tag="t4")
                    nc.vector.tensor_tensor(t4[:], ropein[0][:], s4_sb[:], OP.mult)
                    nc.gpsimd.tensor_tensor(tb[:], tb[:], t4[:], OP.subtract)
                    qk_sb[(tens, g, 0)] = ta
                    qk_sb[(tens, g, 1)] = tb

            # ---- norm scale rows: batched sqrt (single ACT table switch) ----
            sbc = {}
            for g in range(2):
                for tens in range(2):
                    sr = srow.tile([4, TS], BF16, tag="sr")
                    nc.scalar.activation(sr[:],
                                         ssq_ps[g][tens * 32:tens * 32 + 4, :],
                                         AF.Sqrt, bias=eps_sb[0:4, 0:1],
                                         scale=1.0 / DH)
                    with nc.allow_low_precision("scale rows feed bf16 matmul"):
                        nc.vector.reciprocal(sr[:], sr[:])
                    bc_ps = mmps.tile([128, TS], F32, tag="mm", name="bc_ps")
                    nc.tensor.matmul(bc_ps[:], bs4_sb[:], sr[:],
                                     start=True, stop=True)
                    bcs = scr.tile([128, TS], BF16, tag="bcs")
                    nc.scalar.copy(bcs[:], bc_ps[:])
                    sbc[(tens, g)] = bcs

            # ---- rq = relu(q~)*sqb ; ksc = k~*skb ; batched sigmoids ----
            rq = {}
            sig = {}
            for g in range(2):
                for ti in range(4):
                    r = rqp.tile([128, TS], BF16, tag="rq")
                    nc.vector.scalar_tensor_tensor(r[:], qk_sb[(0, g, ti)][:],
                                                   0.0, sbc[(0, g)][:],
                                                   op0=OP.max, op1=OP.mult)
                    rq[(g, ti)] = r
            for g in range(2):
                for ti in range(4):
                    ks = kvp.tile([128, TS], BF16, tag="ksc")
                    nc.vector.tensor_tensor(ks[:], qk_sb[(1, g, ti)][:],
                                            sbc[(1, g)][:], OP.mult)
                    sg = kvp.tile([128, TS], F32, tag="sig")
                    nc.scalar.activation(sg[:], ks[:], AF.Sigmoid)
                    sig[(g, ti)] = sg

            # ---- v / skv / scan / y / cproj1 ----
            for g in range(2):
                for ti in range(4):
                    m = g * 4 + ti
                    vp = mmps.tile([128, TS], F32, tag="mm")
                    for kt in range(2):
                        mm(vp[:], wq_ap(2, m, kt),
                           hT[:, kt * TS:(kt + 1) * TS], kt == 0, kt == 1)
                    skv = sig[(g, ti)]
                    nc.vector.tensor_tensor(skv[:], skv[:], vp[:], OP.mult)
                    nc.vector.tensor_tensor_scan(
                        skv[:], skv[:], skv[:], state[:, m:m + 1],
                        op0=OP.add, op1=OP.bypass)
                    nc.vector.tensor_copy(state[:, m:m + 1], skv[:, TS - 1:TS])
                    y = rq[(g, ti)]
                    nc.gpsimd.tensor_tensor(y[:], y[:], skv[:], OP.mult)
                    for rt in range(2):
                        mm(pp[rt][:],
                           w1c_sb[:, m * 256 + rt * 128:m * 256 + (rt + 1) * 128],
                           y[:], start=(m == 0), stop=(m == 7))

            for rt in range(2):
                p_sb = pbp.tile([128, TS], BF16, tag="pev", name="p_sb")
                nc.scalar.copy(p_sb[:], pp[rt][:])
                nc.sync.dma_start(
                    p_in[s // ns2, rt, :, (s % ns2) * TS:(s % ns2 + 1) * TS],
                    p_sb[:])

        # ================= pair reduce-scatter =================
        nc.gpsimd.collective_compute(
            "ReduceScatter", OP.add,
            replica_groups=[[0, 1], [2, 3], [4, 5], [6, 7]],
            ins=[p_in.opt()], outs=[p_out.opt()])

        # ================= output half =================
        for s2 in range(ns2):
            t0 = s2 * TS
            psb = pbp.tile([128, 2 * TS], BF16, tag="psb")
            for rt in range(2):
                nc.sync.dma_start(psb[:, rt * TS:(rt + 1) * TS],
                                  p_out[rt, :, t0:t0 + TS])
            xrt = []
            outt = []
            for tt in range(TS // 128):
                xr = xpool.tile([128, C], F32, tag="x")
                nc.sync.dma_start(xr[:], xres_d[t0 + tt * 128:t0 + (tt + 1) * 128, :])
                xrt.append(xr)
                outt.append(outp.tile([128, C], F32, tag=f"out{tt}", name=f"out{tt}"))
            for mc in range(8):
                up = mmps.tile([128, TS], F32, tag="mm")
                gp = mmps.tile([128, TS], F32, tag="mm")
                for kt in range(2):
                    mm(up[:], w2c_sb[:, kt * 2048 + mc * 128:kt * 2048 + (mc + 1) * 128],
                       psb[:, kt * TS:(kt + 1) * TS], kt == 0, kt == 1)
                for kt in range(2):
                    mm(gp[:], w2c_sb[:, kt * 2048 + 1024 + mc * 128:kt * 2048 + 1024 + (mc + 1) * 128],
                       psb[:, kt * TS:(kt + 1) * TS], kt == 0, kt == 1)
                sg = usgp.tile([128, TS], F32, tag="sg")
                nc.scalar.activation(sg[:], gp[:], AF.Silu)
                usg = usgp.tile([128, TS], F32, tag="usg")
                nc.vector.tensor_tensor(usg[:], up[:], sg[:], OP.mult)
                for tt in range(TS // 128):
                    tp = mmps.tile([128, 128], F32, tag="mm")
                    nc.tensor.transpose(tp[:], usg[:, tt * 128:(tt + 1) * 128],
                                        ident[:])
                    nc.vector.tensor_tensor(
                        outt[tt][:, mc * 128:(mc + 1) * 128], tp[:],
                        xrt[tt][:, mc * 128:(mc + 1) * 128], OP.add)
            for tt in range(TS // 128):
                nc.sync.dma_start(out_d[t0 + tt * 128:t0 + (tt + 1) * 128, :],
                                  outt[tt][:])


# --------------------------------------------------------------------------
# host wrapper
# --------------------------------------------------------------------------

_NC_CACHE = {}


def _get_nc(t_len, ts):
    if (t_len, ts) not in _NC_CACHE:
        nc = _build(t_len, ts)
        nc.finalize()
        _NC_CACHE[(t_len, ts)] = nc
    return _NC_CACHE[(t_len, ts)]


def _run(x, qkv_w1, qkv_w2, cproj_w1, cproj_w2, t_len, ts=TS_DEFAULT, **spmd_kwargs):
    x = np.asarray(x, np.float32)
    w1 = np.asarray(qkv_w1, np.float32)
    w2 = np.asarray(qkv_w2, np.float32)
    cw1 = np.asarray(cproj_w1, np.float32)
    cw2 = np.asarray(cproj_w2, np.float32)
    nb = x.shape[0]

    import ml_dtypes
    c4, s4 = _tables(t_len)
    sel4 = np.ascontiguousarray(_sel4().astype(ml_dtypes.bfloat16))
    bs4 = np.ascontiguousarray(_sel4().T.astype(ml_dtypes.bfloat16))

    in_maps = []
    for ci in range(NCORES):
        b, hg = ci // 2, ci % 2
        d = _core_arrays(x[b % nb], w1, w2, cw1, cw2, hg, t_len)
        d.update(c4=c4, s4=s4, sel4=sel4, bs4=bs4)
        in_maps.append(d)

    nc = _get_nc(t_len, ts)
    res = bass_utils.run_bass_kernel_spmd(nc, in_maps, list(range(NCORES)),
                                          **spmd_kwargs)
    th = t_len // 2
    out = np.zeros((nb, t_len, C), np.float32)
    for ci in range(NCORES):
        b, hg = ci // 2, ci % 2
        if b < nb:
            out[b, hg * th:(hg + 1) * th, :] = res.results[ci]["out"]
    return out, res


def kernel(x, qkv_w1, qkv_w2, cproj_w1, cproj_w2):
    out, _ = _run(x, qkv_w1, qkv_w2, cproj_w1, cproj_w2, T_FULL)
    return out


# revision 25
# speedup vs baseline: 1.0886x; 1.0886x over previous
"""Trainium2 Bass kernel for nn_AttentionOnDetail (dense transformer block with
low-rank qkv projection, rope, sigmoid-gated causal linear attention, low-rank
gated output projection).

Sharding: 8 cores = (batch b in 0..3) x (head-group hg in 0..1).  Each core
computes its 8 heads over the full sequence, produces a partial cproj_w1
projection p [256, T]; a per-pair ReduceScatter(add) sums the two head-group
partials and hands each core its token-half, on which it runs cproj_w2, the
silu gating and the residual.  Output: each core returns its [T/2, C] slice.

On-core layout: activations flow in [feature, token] layout so every matmul
contraction runs over the partition axis.  The 2048 qkv feature rows are
permuted (host-side weight reordering) so rope-active row blocks are packed
into full 128-partition tiles:
  per 4-head group g, tiles A=[dh 0:32 x4 heads], B=[dh 64:96], C=[dh 32:64],
  D=[dh 96:128]; rope touches only A,B: outA = A*c4 + B*s4, outB = B*c4 - A*s4.
rmsnorm of q/k is computed via selector-matrix matmuls (column sums of
squares on the tensor engine), applied through broadcast-matmul row tiles.
The causal cumsum is the hardware prefix scan (tensor_tensor_scan) along the
free (token) axis, chained across token slices via a carried state column.
"""
import sys
import numpy as np

for _p in ("/opt/trn_rl_repo", "/opt/pypackages"):
    if _p not in sys.path:
        sys.path.insert(0, _p)

import concourse.bass as bass
import concourse.bacc as bacc
import concourse.mybir as mybir
import concourse.tile as tile
from concourse.masks import make_identity
from concourse import bass_utils

F32 = mybir.dt.float32
F32R = mybir.dt.float32r
BF16 = mybir.dt.bfloat16
AF = mybir.ActivationFunctionType
OP = mybir.AluOpType

B, T_FULL, C = 4, 4096, 1024
NH, DH = 16, 128
DQKV, DR = 2048, 256
EPS = float(np.finfo(np.float32).eps)
NCORES = 8
HPG = 4                # heads per packed group
NGRP = 2               # groups per core (8 heads/core)
TS_DEFAULT = 512           # token slice


# --------------------------------------------------------------------------
# host-side packing helpers
# --------------------------------------------------------------------------

def _packed_rows(hg):
    """8 tiles x 128 global dqkv row indices, order g0 A,B,C,D then g1 A,B,C,D."""
    tiles = []
    for g in range(NGRP):
        for dh0 in (0, 64, 32, 96):       # A, B, C, D
            rows = []
            for i in range(HPG):
                h = hg * 8 + g * HPG + i
                rows.extend(h * 128 + dh0 + j for j in range(32))
            tiles.append(rows)
    return tiles


def _tables(t_len):
    quarter = DH // 4
    freq = (1.0 / 1024.0) ** np.linspace(0.0, 1.0, quarter).astype(np.float32)
    pos = np.arange(t_len, dtype=np.float32)
    theta = pos[None, :] * freq[:, None]
    c4 = np.tile(np.cos(theta), (4, 1)).astype(np.float32)   # [128, T]
    s4 = np.tile(np.sin(theta), (4, 1)).astype(np.float32)
    return c4, s4


def _sel4():
    m = np.zeros((128, 4), np.float32)
    for k in range(128):
        m[k, k // 32] = 1.0
    return m


def _core_arrays(x_b, w1, w2, cw1, cw2, hg, t_len):
    tiles = _packed_rows(hg)
    lhsT = []
    for base in (0, DQKV, 2 * DQKV):
        for rows in tiles:
            lhsT.append(w2[np.array(rows) + base, :].T)
    qkv_lhsT = np.ascontiguousarray(np.stack(lhsT))          # [24, 256, 128]
    w1t = np.ascontiguousarray(w1.T)                          # [1024, 256]
    local_cols = np.concatenate([np.array(r) for r in tiles])
    w1c_T = np.ascontiguousarray(cw1[:, local_cols].T)        # [1024, 256]
    w2c_T = np.ascontiguousarray(cw2.T)                       # [256, 2048]
    th = t_len // 2
    xres = np.ascontiguousarray(x_b[hg * th:(hg + 1) * th, :])
    import ml_dtypes
    bf = lambda a: np.ascontiguousarray(a.astype(ml_dtypes.bfloat16))
    return dict(x=np.ascontiguousarray(x_b), xres=xres, wq=bf(qkv_lhsT),
                w1t=bf(w1t), w1c=bf(w1c_T), w2c=bf(w2c_T))


# --------------------------------------------------------------------------
# device kernel
# --------------------------------------------------------------------------

def _build(t_len, TS):
    ns = t_len // TS          # number of token slices
    th = t_len // 2           # this core's output token-half
    ns2 = th // TS            # output slices

    nc = bacc.Bacc("TRN2", target_bir_lowering=False, debug=False,
                   num_devices=NCORES)

    x_d = nc.dram_tensor("x", [t_len, C], F32, kind="ExternalInput").ap()
    xres_d = nc.dram_tensor("xres", [th, C], F32, kind="ExternalInput").ap()
    wq_d = nc.dram_tensor("wq", [24, 256, 128], BF16, kind="ExternalInput").ap()
    w1t_d = nc.dram_tensor("w1t", [C, DR], BF16, kind="ExternalInput").ap()
    w1c_d = nc.dram_tensor("w1c", [1024, DR], BF16, kind="ExternalInput").ap()
    w2c_d = nc.dram_tensor("w2c", [DR, 2 * C], BF16, kind="ExternalInput").ap()
    c4_d = nc.dram_tensor("c4", [128, t_len], F32, kind="ExternalInput").ap()
    s4_d = nc.dram_tensor("s4", [128, t_len], F32, kind="ExternalInput").ap()
    sel4_d = nc.dram_tensor("sel4", [128, 4], BF16, kind="ExternalInput").ap()
    bs4_d = nc.dram_tensor("bs4", [4, 128], BF16, kind="ExternalInput").ap()
    out_d = nc.dram_tensor("out", [th, C], F32, kind="ExternalOutput").ap()

    with tile.TileContext(nc) as tc:
        _body(tc, TS, ns, ns2, th, x_d, xres_d, wq_d, w1t_d, w1c_d, w2c_d,
              c4_d, s4_d, sel4_d, bs4_d, out_d)
    return nc


def _body(tc, TS, ns, ns2, th, x_d, xres_d, wq_d, w1t_d, w1c_d, w2c_d,
          c4_d, s4_d, sel4_d, bs4_d, out_d):
    nc = tc.nc
    from contextlib import ExitStack
    ctx = ExitStack()
    with ctx:
        # ---- pools ----
        wpool = ctx.enter_context(tc.tile_pool(name="weights", bufs=1))
        xpool = ctx.enter_context(tc.tile_pool(name="x", bufs=4))
        xnpool = ctx.enter_context(tc.tile_pool(name="xn", bufs=2))
        xntp = ctx.enter_context(tc.tile_pool(name="xnt", bufs=2))
        hpool = ctx.enter_context(tc.tile_pool(name="h", bufs=2))
        scr = ctx.enter_context(tc.tile_pool(name="scr", bufs=3))
        xsqp = ctx.enter_context(tc.tile_pool(name="xsq", bufs=1))
        ropep = ctx.enter_context(tc.tile_pool(name="rope", bufs=3))
        qkp = ctx.enter_context(tc.tile_pool(name="qk", bufs=18))
        rqp = ctx.enter_context(tc.tile_pool(name="rq", bufs=10))
        kvp = ctx.enter_context(tc.tile_pool(name="kv", bufs=10))
        srow = ctx.enter_context(tc.tile_pool(name="srow", bufs=2))
        small = ctx.enter_context(tc.tile_pool(name="small", bufs=8))
        statep = ctx.enter_context(tc.tile_pool(name="state", bufs=1))
        tabp = ctx.enter_context(tc.tile_pool(name="tab", bufs=2))
        outp = ctx.enter_context(tc.tile_pool(name="outp", bufs=1))
        usgp = ctx.enter_context(tc.tile_pool(name="usg", bufs=2))
        pbp = ctx.enter_context(tc.tile_pool(name="pb", bufs=2))

        mmps = ctx.enter_context(tc.tile_pool(name="mmps", bufs=4, space="PSUM"))
        ppps = ctx.enter_context(tc.tile_pool(name="ppps", bufs=1, space="PSUM"))
        ssqps = ctx.enter_context(tc.tile_pool(name="ssqps", bufs=2, space="PSUM"))

        dram = ctx.enter_context(tc.tile_pool(name="dram", bufs=1, space="DRAM"))

        # ---- static weights ----
        # weights ride the ACT HWDGE queue so slice-0 x loads (SP queue)
        # are not serialized behind 12MB of weight traffic
        wq_sb = wpool.tile([128, 24 * 2 * 128], BF16)
        for m in range(24):
            for kt in range(2):
                blk = m * 2 + kt
                nc.scalar.dma_start(wq_sb[:, blk * 128:(blk + 1) * 128],
                                    wq_d[m, kt * 128:(kt + 1) * 128, :])
        w1t_sb = wpool.tile([128, 8 * 256], BF16)
        w1c_sb = wpool.tile([128, 8 * 256], BF16)
        for kt in range(8):
            nc.scalar.dma_start(w1t_sb[:, kt * 256:(kt + 1) * 256],
                                w1t_d[kt * 128:(kt + 1) * 128, :])
            nc.scalar.dma_start(w1c_sb[:, kt * 256:(kt + 1) * 256],
                                w1c_d[kt * 128:(kt + 1) * 128, :])
        w2c_sb = wpool.tile([128, 2 * 2048], BF16)
        for kt in range(2):
            nc.scalar.dma_start(w2c_sb[:, kt * 2048:(kt + 1) * 2048],
                                w2c_d[kt * 128:(kt + 1) * 128, :])
        sel4_sb = wpool.tile([128, 4], BF16)
        nc.scalar.dma_start(sel4_sb[:], sel4_d)
        bs4_sb = wpool.tile([4, 128], BF16)
        nc.scalar.dma_start(bs4_sb[:], bs4_d)
        ident = wpool.tile([128, 128], F32)
        make_identity(nc, ident)
        eps_sb = wpool.tile([128, 1], F32)
        nc.vector.memset(eps_sb[:], EPS)

        state = statep.tile([128, 8], F32)
        nc.vector.memset(state[:], 0.0)

        p_in = [dram.tile([2, 2, 128, TS], BF16, name=f"p_in{j}")
                for j in range(ns2)]
        p_out = [dram.tile([2, 128, TS], BF16, name=f"p_out{j}")
                 for j in range(ns2)]

        def mm(out, lhsT, rhs, start, stop):
            nc.tensor.matmul(out, lhsT, rhs, start=start, stop=stop)

        # q/k/v lhsT slice for packed M-tile m of tensor `tens`, K-tile kt
        def wq_ap(tens, m, kt):
            base = (tens * 8 + m) * 2 + kt
            return wq_sb[:, base * 128:(base + 1) * 128]

        # ================= main slice loop =================
        for s in range(ns):
            t0 = s * TS
            # ---- tables for this slice ----
            c4_sb = tabp.tile([128, TS], F32, tag="c4")
            nc.sync.dma_start(c4_sb[:], c4_d[:, t0:t0 + TS])
            s4_sb = tabp.tile([128, TS], F32, tag="s4")
            nc.sync.dma_start(s4_sb[:], s4_d[:, t0:t0 + TS])

            # ---- x-prep: rmsnorm + transpose ----
            xnT = xntp.tile([128, 8 * TS], BF16, tag="xnT")
            for tt in range(TS // 128):
                xt = xpool.tile([128, C], F32, tag="x")
                nc.sync.dma_start(xt[:], x_d[t0 + tt * 128:t0 + (tt + 1) * 128, :])
                xsq = xsqp.tile([128, C], F32, tag="xsq")
                ssqc = small.tile([128, 1], F32, tag="ssqc")
                nc.scalar.activation(xsq[:], xt[:], AF.Square, accum_out=ssqc[:])
                rms = small.tile([128, 1], F32, tag="rms")
                nc.scalar.activation(rms[:], ssqc[:], AF.Sqrt,
                                     bias=eps_sb[:, 0:1], scale=1.0 / C)
                rstd = small.tile([128, 1], F32, tag="rstd")
                nc.vector.reciprocal(rstd[:], rms[:])
                xn = xnpool.tile([128, C], F32, tag="xn")
                nc.vector.tensor_scalar_mul(xn[:], xt[:], rstd[:, 0:1])
                for ch in range(2):
                    tp = mmps.tile([128, 512], F32, tag="mm", name="tp")
                    for cc in range(4):
                        nc.tensor.transpose(
                            tp[:, cc * 128:(cc + 1) * 128],
                            xn[:, (ch * 4 + cc) * 128:(ch * 4 + cc + 1) * 128],
                            ident[:])
                    # one strided evacuation per 4 transposed blocks
                    nc.vector.tensor_copy(
                        xnT.rearrange("p (c t) -> p c t", c=8)
                           [:, ch * 4:(ch + 1) * 4, tt * 128:(tt + 1) * 128],
                        tp.rearrange("p (c t) -> p c t", c=4))

            # ---- h = relu(rmsnorm(x) @ w1.T)^2, in [DR, TS] layout ----
            hT = hpool.tile([128, 2 * TS], BF16, tag="hT")
            for mt in range(2):
                hp = mmps.tile([128, TS], F32, tag="mm")
                for kt in range(8):
                    mm(hp[:], w1t_sb[:, kt * 256 + mt * 128:kt * 256 + (mt + 1) * 128],
                       xnT[:, kt * TS:(kt + 1) * TS], kt == 0, kt == 7)
                hc = scr.tile([128, TS], F32, tag="hc")
                nc.scalar.copy(hc[:], hp[:])
                nc.vector.scalar_tensor_tensor(
                    hT[:, mt * TS:(mt + 1) * TS], hc[:], 0.0, hc[:],
                    op0=OP.max, op1=OP.mult)

            # ---- attention: qk matmuls + squares + colsums + rope ----
            # (ACT uses only Square/Copy here -- both live in every table set)
            pp = [ppps.tile([128, TS], F32, tag=f"pp{rt}", name=f"pp{rt}")
                  for rt in range(2)]
            qk_sb = {}
            ssq_ps = {}
            for g in range(2):
                ssq_t = ssqps.tile([36, TS], F32, tag="ssq", name=f"ssq{g}")
                ssq_ps[g] = ssq_t
                for tens in range(2):          # 0 = q, 1 = k
                    ropein = {}
                    for ti in range(4):
                        qp = mmps.tile([128, TS], F32, tag="mm")
                        for kt in range(2):
                            mm(qp[:], wq_ap(tens, g * 4 + ti, kt),
                               hT[:, kt * TS:(kt + 1) * TS], kt == 0, kt == 1)
                        sqt = scr.tile([128, TS], BF16, tag="sqt")
                        nc.scalar.square(sqt[:], qp[:])
                        nc.tensor.matmul(ssq_t[tens * 32:tens * 32 + 4, :],
                                         sel4_sb[:], sqt[:],
                                         start=(ti == 0), stop=(ti == 3))
                        if ti < 2:
                            ropein[ti] = qp
                        else:
                            cd = qkp.tile([128, TS], BF16, tag="qk", name="cd")
                            nc.scalar.copy(cd[:], qp[:])
                            qk_sb[(tens, g, ti)] = cd
                    # rope: A = z1*c + z2*s ; B = z2*c - z1*s
                    ta = qkp.tile([128, TS], BF16, tag="qk", name="ta")
                    nc.vector.tensor_tensor(ta[:], ropein[0][:], c4_sb[:], OP.mult)
                    t2 = ropep.tile([128, TS], BF16, tag="t2")
                    nc.vector.tensor_tensor(t2[:], ropein[1][:], s4_sb[:], OP.mult)
                    nc.gpsimd.tensor_tensor(ta[:], ta[:], t2[:], OP.add)
                    tb = qkp.tile([128, TS], BF16, tag="qk", name="tb")
                    nc.vector.tensor_tensor(tb[:], ropein[1][:], c4_sb[:], OP.mult)
                    t4 = ropep.tile([128, TS], BF16, tag="t4")
                    nc.vector.tensor_tensor(t4[:], ropein[0][:], s4_sb[:], OP.mult)
                    nc.gpsimd.tensor_tensor(tb[:], tb[:], t4[:], OP.subtract)
                    qk_sb[(tens, g, 0)] = ta
                    qk_sb[(tens, g, 1)] = tb

            # ---- norm scale rows: batched sqrt (single ACT table switch) ----
            sbc = {}
            for g in range(2):
                for tens in range(2):
                    sr = srow.tile([4, TS], BF16, tag="sr")
                    nc.scalar.activation(sr[:],
                                         ssq_ps[g][tens * 32:tens * 32 + 4, :],
                                         AF.Sqrt, bias=eps_sb[0:4, 0:1],
                                         scale=1.0 / DH)
                    with nc.allow_low_precision("scale rows feed bf16 matmul"):
                        nc.vector.reciprocal(sr[:], sr[:])
                    bc_ps = mmps.tile([128, TS], F32, tag="mm", name="bc_ps")
                    nc.tensor.matmul(bc_ps[:], bs4_sb[:], sr[:],
                                     start=True, stop=True)
                    bcs = scr.tile([128, TS], BF16, tag="bcs")
                    nc.scalar.copy(bcs[:], bc_ps[:])
                    sbc[(tens, g)] = bcs

            # ---- rq = relu(q~)*sqb ; ksc = k~*skb ; batched sigmoids ----
            rq = {}
            sig = {}
            for g in range(2):
                for ti in range(4):
                    r = rqp.tile([128, TS], BF16, tag="rq")
                    nc.vector.scalar_tensor_tensor(r[:], qk_sb[(0, g, ti)][:],
                                                   0.0, sbc[(0, g)][:],
                                                   op0=OP.max, op1=OP.mult)
                    rq[(g, ti)] = r
            for g in range(2):
                for ti in range(4):
                    ks = kvp.tile([128, TS], BF16, tag="ksc")
                    nc.vector.tensor_tensor(ks[:], qk_sb[(1, g, ti)][:],
                                            sbc[(1, g)][:], OP.mult)
                    sg = kvp.tile([128, TS], F32, tag="sig")
                    nc.scalar.activation(sg[:], ks[:], AF.Sigmoid)
                    sig[(g, ti)] = sg

            # ---- v / skv / scan / y / cproj1 ----
            for g in range(2):
                for ti in range(4):
                    m = g * 4 + ti
                    vp = mmps.tile([128, TS], F32, tag="mm")
                    for kt in range(2):
                        mm(vp[:], wq_ap(2, m, kt),
                           hT[:, kt * TS:(kt + 1) * TS], kt == 0, kt == 1)
                    skv = sig[(g, ti)]
                    nc.vector.tensor_tensor(skv[:], skv[:], vp[:], OP.mult)
                    nc.vector.tensor_tensor_scan(
                        skv[:], skv[:], skv[:], state[:, m:m + 1],
                        op0=OP.add, op1=OP.bypass)
                    nc.vector.tensor_copy(state[:, m:m + 1], skv[:, TS - 1:TS])
                    y = rq[(g, ti)]
                    nc.gpsimd.tensor_tensor(y[:], y[:], skv[:], OP.mult)
                    for rt in range(2):
                        mm(pp[rt][:],
                           w1c_sb[:, m * 256 + rt * 128:m * 256 + (rt + 1) * 128],
                           y[:], start=(m == 0), stop=(m == 7))

            for rt in range(2):
                p_sb = pbp.tile([128, TS], BF16, tag="pev", name="p_sb")
                nc.scalar.copy(p_sb[:], pp[rt][:])
                nc.sync.dma_start(p_in[s % ns2][s // ns2, rt, :, :], p_sb[:])

        # ================= pair reduce-scatter, chunked for overlap =========
        for j in range(ns2):
            nc.gpsimd.collective_compute(
                "ReduceScatter", OP.add,
                replica_groups=[[0, 1], [2, 3], [4, 5], [6, 7]],
                ins=[p_in[j].opt()], outs=[p_out[j].opt()])

        # ================= output half =================
        for s2 in range(ns2):
            t0 = s2 * TS
            psb = pbp.tile([128, 2 * TS], BF16, tag="psb")
            for rt in range(2):
                nc.sync.dma_start(psb[:, rt * TS:(rt + 1) * TS],
                                  p_out[s2][rt, :, :])
            xrt = []
            outt = []
            for tt in range(TS // 128):
                xr = xpool.tile([128, C], F32, tag="x")
                nc.sync.dma_start(xr[:], xres_d[t0 + tt * 128:t0 + (tt + 1) * 128, :])
                xrt.append(xr)
                outt.append(outp.tile([128, C], F32, tag=f"out{tt}", name=f"out{tt}"))
            for mc in range(8):
                up = mmps.tile([128, TS], F32, tag="mm")
                gp = mmps.tile([128, TS], F32, tag="mm")
                for kt in range(2):
                    mm(up[:], w2c_sb[:, kt * 2048 + mc * 128:kt * 2048 + (mc + 1) * 128],
                       psb[:, kt * TS:(kt + 1) * TS], kt == 0, kt == 1)
                for kt in range(2):
                    mm(gp[:], w2c_sb[:, kt * 2048 + 1024 + mc * 128:kt * 2048 + 1024 + (mc + 1) * 128],
                       psb[:, kt * TS:(kt + 1) * TS], kt == 0, kt == 1)
                sg = usgp.tile([128, TS], F32, tag="sg")
                nc.scalar.activation(sg[:], gp[:], AF.Silu)
                usg = usgp.tile([128, TS], F32, tag="usg")
                nc.vector.tensor_tensor(usg[:], up[:], sg[:], OP.mult)
                for tt in range(TS // 128):
                    tp = mmps.tile([128, 128], F32, tag="mm")
                    nc.tensor.transpose(tp[:], usg[:, tt * 128:(tt + 1) * 128],
                                        ident[:])
                    nc.vector.tensor_tensor(
                        outt[tt][:, mc * 128:(mc + 1) * 128], tp[:],
                        xrt[tt][:, mc * 128:(mc + 1) * 128], OP.add)
            for tt in range(TS // 128):
                nc.sync.dma_start(out_d[t0 + tt * 128:t0 + (tt + 1) * 128, :],
                                  outt[tt][:])


# --------------------------------------------------------------------------
# host wrapper
# --------------------------------------------------------------------------

_NC_CACHE = {}


def _get_nc(t_len, ts):
    if (t_len, ts) not in _NC_CACHE:
        nc = _build(t_len, ts)
        nc.finalize()
        _NC_CACHE[(t_len, ts)] = nc
    return _NC_CACHE[(t_len, ts)]


def _run(x, qkv_w1, qkv_w2, cproj_w1, cproj_w2, t_len, ts=TS_DEFAULT, **spmd_kwargs):
    x = np.asarray(x, np.float32)
    w1 = np.asarray(qkv_w1, np.float32)
    w2 = np.asarray(qkv_w2, np.float32)
    cw1 = np.asarray(cproj_w1, np.float32)
    cw2 = np.asarray(cproj_w2, np.float32)
    nb = x.shape[0]

    import ml_dtypes
    c4, s4 = _tables(t_len)
    sel4 = np.ascontiguousarray(_sel4().astype(ml_dtypes.bfloat16))
    bs4 = np.ascontiguousarray(_sel4().T.astype(ml_dtypes.bfloat16))

    in_maps = []
    for ci in range(NCORES):
        b, hg = ci // 2, ci % 2
        d = _core_arrays(x[b % nb], w1, w2, cw1, cw2, hg, t_len)
        d.update(c4=c4, s4=s4, sel4=sel4, bs4=bs4)
        in_maps.append(d)

    nc = _get_nc(t_len, ts)
    res = bass_utils.run_bass_kernel_spmd(nc, in_maps, list(range(NCORES)),
                                          **spmd_kwargs)
    th = t_len // 2
    out = np.zeros((nb, t_len, C), np.float32)
    for ci in range(NCORES):
        b, hg = ci // 2, ci % 2
        if b < nb:
            out[b, hg * th:(hg + 1) * th, :] = res.results[ci]["out"]
    return out, res


def kernel(x, qkv_w1, qkv_w2, cproj_w1, cproj_w2):
    out, _ = _run(x, qkv_w1, qkv_w2, cproj_w1, cproj_w2, T_FULL)
    return out


# revision 56
# speedup vs baseline: 1.2108x; 1.1122x over previous
"""Trainium2 Bass kernel for nn_AttentionOnDetail (dense transformer block with
low-rank qkv projection, rope, sigmoid-gated causal linear attention, low-rank
gated output projection).

Sharding: 8 cores = (batch b in 0..3) x (head-group hg in 0..1).  Each core
computes its 8 heads over the full sequence, produces a partial cproj_w1
projection p [256, T]; a per-pair ReduceScatter(add) sums the two head-group
partials and hands each core its token-half, on which it runs cproj_w2, the
silu gating and the residual.  Output: each core returns its [T/2, C] slice.

On-core layout: activations flow in [feature, token] layout so every matmul
contraction runs over the partition axis.  The 2048 qkv feature rows are
permuted (host-side weight reordering) so rope-active row blocks are packed
into full 128-partition tiles:
  per 4-head group g, tiles A=[dh 0:32 x4 heads], B=[dh 64:96], C=[dh 32:64],
  D=[dh 96:128]; rope touches only A,B: outA = A*c4 + B*s4, outB = B*c4 - A*s4.
rmsnorm of q/k is computed via selector-matrix matmuls (column sums of
squares on the tensor engine), applied through broadcast-matmul row tiles.
The causal cumsum is the hardware prefix scan (tensor_tensor_scan) along the
free (token) axis, chained across token slices via a carried state column.
"""
import sys
import numpy as np

for _p in ("/opt/trn_rl_repo", "/opt/pypackages"):
    if _p not in sys.path:
        sys.path.insert(0, _p)

import concourse.bass as bass
import concourse.bacc as bacc
import concourse.mybir as mybir
import concourse.tile as tile
from concourse.masks import make_identity
from concourse import bass_utils

F32 = mybir.dt.float32
F32R = mybir.dt.float32r
BF16 = mybir.dt.bfloat16
AF = mybir.ActivationFunctionType
OP = mybir.AluOpType

B, T_FULL, C = 4, 4096, 1024
NH, DH = 16, 128
DQKV, DR = 2048, 256
EPS = float(np.finfo(np.float32).eps)
NCORES = 8
HPG = 4                # heads per packed group
NGRP = 2               # groups per core (8 heads/core)
TS_DEFAULT = 512           # token slice


# --------------------------------------------------------------------------
# host-side packing helpers
# --------------------------------------------------------------------------

def _packed_rows(hg):
    """8 tiles x 128 global dqkv row indices, order g0 A,B,C,D then g1 A,B,C,D."""
    tiles = []
    for g in range(NGRP):
        for dh0 in (0, 64, 32, 96):       # A, B, C, D
            rows = []
            for i in range(HPG):
                h = hg * 8 + g * HPG + i
                rows.extend(h * 128 + dh0 + j for j in range(32))
            tiles.append(rows)
    return tiles


def _tables(t_len):
    quarter = DH // 4
    freq = (1.0 / 1024.0) ** np.linspace(0.0, 1.0, quarter).astype(np.float32)
    pos = np.arange(t_len, dtype=np.float32)
    theta = pos[None, :] * freq[:, None]
    c4 = np.tile(np.cos(theta), (4, 1)).astype(np.float32)   # [128, T]
    s4 = np.tile(np.sin(theta), (4, 1)).astype(np.float32)
    return c4, s4


def _sel4():
    m = np.zeros((128, 4), np.float32)
    for k in range(128):
        m[k, k // 32] = 1.0
    return m


def _core_arrays(x_b, w1, w2, cw1, cw2, hg, t_len):
    tiles = _packed_rows(hg)
    lhsT = []
    for base in (0, DQKV, 2 * DQKV):
        for rows in tiles:
            lhsT.append(w2[np.array(rows) + base, :].T)
    qkv_lhsT = np.ascontiguousarray(np.stack(lhsT))          # [24, 256, 128]
    w1t = np.ascontiguousarray(w1.T)                          # [1024, 256]
    local_cols = np.concatenate([np.array(r) for r in tiles])
    w1c_T = np.ascontiguousarray(cw1[:, local_cols].T)        # [1024, 256]
    w2c_T = np.ascontiguousarray(cw2.T)                       # [256, 2048]
    th = t_len // 2
    xres = np.ascontiguousarray(x_b[hg * th:(hg + 1) * th, :])
    import ml_dtypes
    bf = lambda a: np.ascontiguousarray(a.astype(ml_dtypes.bfloat16))
    return dict(x=np.ascontiguousarray(x_b), xres=xres, wq=bf(qkv_lhsT),
                w1t=bf(w1t), w1c=bf(w1c_T), w2c=bf(w2c_T))


# --------------------------------------------------------------------------
# device kernel
# --------------------------------------------------------------------------

def _build(t_len, TS):
    ns = t_len // TS          # number of token slices
    th = t_len // 2           # this core's output token-half
    ns2 = th // TS            # output slices

    nc = bacc.Bacc("TRN2", target_bir_lowering=False, debug=False,
                   num_devices=NCORES)

    x_d = nc.dram_tensor("x", [t_len, C], F32, kind="ExternalInput").ap()
    xres_d = nc.dram_tensor("xres", [th, C], F32, kind="ExternalInput").ap()
    wq_d = nc.dram_tensor("wq", [24, 256, 128], BF16, kind="ExternalInput").ap()
    w1t_d = nc.dram_tensor("w1t", [C, DR], BF16, kind="ExternalInput").ap()
    w1c_d = nc.dram_tensor("w1c", [1024, DR], BF16, kind="ExternalInput").ap()
    w2c_d = nc.dram_tensor("w2c", [DR, 2 * C], BF16, kind="ExternalInput").ap()
    c4_d = nc.dram_tensor("c4", [128, t_len], F32, kind="ExternalInput").ap()
    s4_d = nc.dram_tensor("s4", [128, t_len], F32, kind="ExternalInput").ap()
    sel4_d = nc.dram_tensor("sel4", [128, 4], BF16, kind="ExternalInput").ap()
    bs4_d = nc.dram_tensor("bs4", [4, 128], BF16, kind="ExternalInput").ap()
    out_d = nc.dram_tensor("out", [th, C], F32, kind="ExternalOutput").ap()

    with tile.TileContext(nc) as tc:
        _body(tc, TS, ns, ns2, th, x_d, xres_d, wq_d, w1t_d, w1c_d, w2c_d,
              c4_d, s4_d, sel4_d, bs4_d, out_d)
    return nc


def _body(tc, TS, ns, ns2, th, x_d, xres_d, wq_d, w1t_d, w1c_d, w2c_d,
          c4_d, s4_d, sel4_d, bs4_d, out_d):
    nc = tc.nc
    from contextlib import ExitStack
    ctx = ExitStack()
    with ctx:
        # ---- pools ----
        wpool = ctx.enter_context(tc.tile_pool(name="weights", bufs=1))
        xpool = ctx.enter_context(tc.tile_pool(name="x", bufs=4))
        xnpool = ctx.enter_context(tc.tile_pool(name="xn", bufs=2))
        xntp = ctx.enter_context(tc.tile_pool(name="xnt", bufs=2))
        hpool = ctx.enter_context(tc.tile_pool(name="h", bufs=2))
        scr = ctx.enter_context(tc.tile_pool(name="scr", bufs=3))
        xsqp = ctx.enter_context(tc.tile_pool(name="xsq", bufs=1))
        ropep = ctx.enter_context(tc.tile_pool(name="rope", bufs=3))
        qkp = ctx.enter_context(tc.tile_pool(name="qk", bufs=18))
        rqp = ctx.enter_context(tc.tile_pool(name="rq", bufs=10))
        kvp = ctx.enter_context(tc.tile_pool(name="kv", bufs=10))
        srow = ctx.enter_context(tc.tile_pool(name="srow", bufs=2))
        small = ctx.enter_context(tc.tile_pool(name="small", bufs=8))
        statep = ctx.enter_context(tc.tile_pool(name="state", bufs=1))
        tabp = ctx.enter_context(tc.tile_pool(name="tab", bufs=2))
        outp = ctx.enter_context(tc.tile_pool(name="outp", bufs=1))
        usgp = ctx.enter_context(tc.tile_pool(name="usg", bufs=3))
        pbp = ctx.enter_context(tc.tile_pool(name="pb", bufs=3))

        mmps = ctx.enter_context(tc.tile_pool(name="mmps", bufs=4, space="PSUM"))
        ppps = ctx.enter_context(tc.tile_pool(name="ppps", bufs=1, space="PSUM"))
        ssqps = ctx.enter_context(tc.tile_pool(name="ssqps", bufs=2, space="PSUM"))

        dram = ctx.enter_context(tc.tile_pool(name="dram", bufs=1, space="DRAM"))

        # ---- static weights ----
        # prefetch slice-0 activations ahead of the 12MB weight traffic so
        # the compute engines can start within a few us of kernel entry
        pre_x = []
        for tt in range(TS // 128):
            xt0 = xpool.tile([128, C], F32, tag="x", name="xt0")
            nc.sync.dma_start(xt0[:], x_d[tt * 128:(tt + 1) * 128, :])
            pre_x.append(xt0)
        pre_c4 = tabp.tile([128, TS], F32, tag="c4", name="pre_c4")
        nc.sync.dma_start(pre_c4[:], c4_d[:, 0:TS])
        pre_s4 = tabp.tile([128, TS], F32, tag="s4", name="pre_s4")
        nc.sync.dma_start(pre_s4[:], s4_d[:, 0:TS])

        # weights ride the ACT HWDGE queue so slice-0 x loads (SP queue)
        # are not serialized behind 12MB of weight traffic
        wq_sb = wpool.tile([128, 24 * 2 * 128], BF16)
        for m in range(24):
            for kt in range(2):
                blk = m * 2 + kt
                nc.scalar.dma_start(wq_sb[:, blk * 128:(blk + 1) * 128],
                                    wq_d[m, kt * 128:(kt + 1) * 128, :])
        w1t_sb = wpool.tile([128, 8 * 256], BF16)
        w1c_sb = wpool.tile([128, 8 * 256], BF16)
        for kt in range(8):
            nc.scalar.dma_start(w1t_sb[:, kt * 256:(kt + 1) * 256],
                                w1t_d[kt * 128:(kt + 1) * 128, :])
            nc.scalar.dma_start(w1c_sb[:, kt * 256:(kt + 1) * 256],
                                w1c_d[kt * 128:(kt + 1) * 128, :])
        w2c_sb = wpool.tile([128, 2 * 2048], BF16)
        for kt in range(2):
            nc.scalar.dma_start(w2c_sb[:, kt * 2048:(kt + 1) * 2048],
                                w2c_d[kt * 128:(kt + 1) * 128, :])
        sel4_sb = wpool.tile([128, 4], BF16)
        nc.scalar.dma_start(sel4_sb[:], sel4_d)
        bs4_sb = wpool.tile([4, 128], BF16)
        nc.scalar.dma_start(bs4_sb[:], bs4_d)
        ident = wpool.tile([128, 128], F32)
        make_identity(nc, ident)
        ident_bf = wpool.tile([128, 128], BF16)
        make_identity(nc, ident_bf)
        eps_sb = wpool.tile([128, 1], F32)
        nc.vector.memset(eps_sb[:], EPS)

        state = statep.tile([128, 8], F32)
        nc.vector.memset(state[:], 0.0)

        p_in = [dram.tile([2, 2, 128, TS], BF16, name=f"p_in{j}")
                for j in range(ns2)]
        p_out = [dram.tile([2, 128, TS], BF16, name=f"p_out{j}")
                 for j in range(ns2)]

        def mm(out, lhsT, rhs, start, stop):
            nc.tensor.matmul(out, lhsT, rhs, start=start, stop=stop)

        def act_rsqrt(out, in_, bias_ap, scale):
            eng = nc.scalar
            ins = [eng.lower_ap(in_), eng.lower_ap(bias_ap),
                   mybir.ImmediateValue(dtype=F32, value=scale),
                   mybir.ImmediateValue(dtype=F32, value=0.0)]
            eng.add_instruction(mybir.InstActivation(
                name=nc.get_next_instruction_name(),
                func=AF.Rsqrt, ins=ins, outs=[eng.lower_ap(out)]))

        # q/k/v lhsT slice for packed M-tile m of tensor `tens`, K-tile kt
        def wq_ap(tens, m, kt):
            base = (tens * 8 + m) * 2 + kt
            return wq_sb[:, base * 128:(base + 1) * 128]

        def _emit_output_chunk(s2):
            t0 = s2 * TS
            psb = pbp.tile([128, 2 * TS], BF16, tag="psb")
            for rt in range(2):
                nc.sync.dma_start(psb[:, rt * TS:(rt + 1) * TS],
                                  p_out[s2][rt, :, :])
            xrt = []
            outt = []
            for tt in range(TS // 128):
                xr = xpool.tile([128, C], F32, tag="x", name="xr")
                nc.sync.dma_start(xr[:], xres_d[t0 + tt * 128:t0 + (tt + 1) * 128, :])
                xrt.append(xr)
                outt.append(outp.tile([128, C], F32, tag=f"out{tt}", name=f"out{tt}"))
            for mc in range(8):
                up = mmps.tile([128, TS], F32, tag="mm", name="up")
                gp = mmps.tile([128, TS], F32, tag="mm", name="gp")
                for kt in range(2):
                    mm(up[:], w2c_sb[:, kt * 2048 + mc * 128:kt * 2048 + (mc + 1) * 128],
                       psb[:, kt * TS:(kt + 1) * TS], kt == 0, kt == 1)
                for kt in range(2):
                    mm(gp[:], w2c_sb[:, kt * 2048 + 1024 + mc * 128:kt * 2048 + 1024 + (mc + 1) * 128],
                       psb[:, kt * TS:(kt + 1) * TS], kt == 0, kt == 1)
                sg = usgp.tile([128, TS], F32, tag="sg", name="sg")
                nc.scalar.activation(sg[:], gp[:], AF.Silu)
                usg = usgp.tile([128, TS], F32, tag="usg", name="usg")
                nc.vector.tensor_tensor(usg[:], up[:], sg[:], OP.mult)
                tpb = mmps.tile([128, TS], F32, tag="mm", name="tpb")
                for tt in range(TS // 128):
                    nc.tensor.transpose(tpb[:, tt * 128:(tt + 1) * 128],
                                        usg[:, tt * 128:(tt + 1) * 128],
                                        ident[:])
                for tt in range(TS // 128):
                    nc.vector.tensor_tensor(
                        outt[tt][:, mc * 128:(mc + 1) * 128],
                        tpb[:, tt * 128:(tt + 1) * 128],
                        xrt[tt][:, mc * 128:(mc + 1) * 128], OP.add)
            for tt in range(TS // 128):
                nc.sync.dma_start(out_d[t0 + tt * 128:t0 + (tt + 1) * 128, :],
                                  outt[tt][:])

        # ================= main slice loop =================
        for s in range(ns):
            t0 = s * TS
            # ---- tables for this slice ----
            if s == 0:
                c4_sb, s4_sb = pre_c4, pre_s4
            else:
                c4_sb = tabp.tile([128, TS], F32, tag="c4")
                nc.sync.dma_start(c4_sb[:], c4_d[:, t0:t0 + TS])
                s4_sb = tabp.tile([128, TS], F32, tag="s4")
                nc.sync.dma_start(s4_sb[:], s4_d[:, t0:t0 + TS])

            # ---- x-prep: rmsnorm + transpose ----
            xnT = xntp.tile([128, 8 * TS], BF16, tag="xnT")
            for tt in range(TS // 128):
                if s == 0:
                    xt = pre_x[tt]
                else:
                    xt = xpool.tile([128, C], F32, tag="x")
                    nc.sync.dma_start(xt[:], x_d[t0 + tt * 128:t0 + (tt + 1) * 128, :])
                xsq = xsqp.tile([128, C], F32, tag="xsq")
                ssqc = small.tile([128, 1], F32, tag="ssqc")
                nc.scalar.activation(xsq[:], xt[:], AF.Square, accum_out=ssqc[:])
                rstd = small.tile([128, 1], F32, tag="rstd")
                act_rsqrt(rstd[:], ssqc[:], eps_sb[:, 0:1], 1.0 / C)
                xn = xnpool.tile([128, C], BF16, tag="xn")
                nc.vector.tensor_scalar_mul(xn[:], xt[:], rstd[:, 0:1])
                for ch in range(2):
                    tp = mmps.tile([128, 512], BF16, tag="mm", name="tp")
                    for cc in range(4):
                        nc.tensor.transpose(
                            tp[:, cc * 128:(cc + 1) * 128],
                            xn[:, (ch * 4 + cc) * 128:(ch * 4 + cc + 1) * 128],
                            ident_bf[:])
                    # evacuate on ACT: DVE is the bottleneck engine
                    nc.scalar.copy(
                        xnT.rearrange("p (c t) -> p c t", c=8)
                           [:, ch * 4:(ch + 1) * 4, tt * 128:(tt + 1) * 128],
                        tp.rearrange("p (c t) -> p c t", c=4))

            # ---- h = relu(rmsnorm(x) @ w1.T)^2, in [DR, TS] layout ----
            hT = hpool.tile([128, 2 * TS], BF16, tag="hT")
            for mt in range(2):
                hp = mmps.tile([128, TS], F32, tag="mm")
                for kt in range(8):
                    mm(hp[:], w1t_sb[:, kt * 256 + mt * 128:kt * 256 + (mt + 1) * 128],
                       xnT[:, kt * TS:(kt + 1) * TS], kt == 0, kt == 7)
                hc = scr.tile([128, TS], F32, tag="hc")
                nc.vector.tensor_copy(hc[:], hp[:])
                nc.vector.scalar_tensor_tensor(
                    hT[:, mt * TS:(mt + 1) * TS], hc[:], 0.0, hc[:],
                    op0=OP.max, op1=OP.mult)

            # ---- attention: qk matmuls + squares + colsums + rope ----
            # (ACT uses only Square/Copy here -- both live in every table set)
            pp = [ppps.tile([128, TS], F32, tag=f"pp{rt}", name=f"pp{rt}")
                  for rt in range(2)]
            qk_sb = {}
            ssq_ps = {}
            for g in range(2):
                ssq_t = ssqps.tile([36, TS], F32, tag="ssq", name=f"ssq{g}")
                ssq_ps[g] = ssq_t
                for tens in range(2):          # 0 = q, 1 = k
                    ropein = {}
                    for ti in range(4):
                        qp = mmps.tile([128, TS], F32, tag="mm")
                        for kt in range(2):
                            mm(qp[:], wq_ap(tens, g * 4 + ti, kt),
                               hT[:, kt * TS:(kt + 1) * TS], kt == 0, kt == 1)
                        sqt = scr.tile([128, TS], BF16, tag="sqt")
                        if ti < 2:
                            nc.scalar.square(sqt[:], qp[:])
                            ropein[ti] = qp
                        else:
                            cd = qkp.tile([128, TS], BF16, tag="qk", name="cd")
                            nc.scalar.copy(cd[:], qp[:])
                            qk_sb[(tens, g, ti)] = cd
                            nc.vector.tensor_tensor(sqt[:], cd[:], cd[:], OP.mult)
                        nc.tensor.matmul(ssq_t[tens * 32:tens * 32 + 4, :],
                                         sel4_sb[:], sqt[:],
                                         start=(ti == 0), stop=(ti == 3))
                    # rope: A = z1*c + z2*s ; B = z2*c - z1*s
                    ta = qkp.tile([128, TS], BF16, tag="qk", name="ta")
                    nc.vector.tensor_tensor(ta[:], ropein[0][:], c4_sb[:], OP.mult)
                    t2 = ropep.tile([128, TS], BF16, tag="t2")
                    nc.vector.tensor_tensor(t2[:], ropein[1][:], s4_sb[:], OP.mult)
                    nc.gpsimd.tensor_tensor(ta[:], ta[:], t2[:], OP.add)
                    tb = qkp.tile([128, TS], BF16, tag="qk", name="tb")
                    nc.vector.tensor_tensor(tb[:], ropein[1][:], c4_sb[:], OP.mult)
                    t4 = ropep.tile([128, TS], BF16, tag="t4")
                    nc.vector.tensor_tensor(t4[:], ropein[0][:], s4_sb[:], OP.mult)
                    nc.gpsimd.tensor_tensor(tb[:], tb[:], t4[:], OP.subtract)
                    qk_sb[(tens, g, 0)] = ta
                    qk_sb[(tens, g, 1)] = tb

            # ---- norm scale rows: batched sqrt (single ACT table switch) ----
            sbc = {}
            for g in range(2):
                for tens in range(2):
                    sr = srow.tile([4, TS], BF16, tag="sr")
                    act_rsqrt(sr[:], ssq_ps[g][tens * 32:tens * 32 + 4, :],
                              eps_sb[0:4, 0:1], 1.0 / DH)
                    bc_ps = mmps.tile([128, TS], F32, tag="mm", name="bc_ps")
                    nc.tensor.matmul(bc_ps[:], bs4_sb[:], sr[:],
                                     start=True, stop=True)
                    bcs = scr.tile([128, TS], BF16, tag="bcs")
                    nc.scalar.copy(bcs[:], bc_ps[:])
                    sbc[(tens, g)] = bcs

            # ---- rq = relu(q~)*sqb ; ksc = k~*skb ; batched sigmoids ----
            rq = {}
            sig = {}
            for g in range(2):
                for ti in range(4):
                    r = rqp.tile([128, TS], BF16, tag="rq")
                    nc.vector.scalar_tensor_tensor(r[:], qk_sb[(0, g, ti)][:],
                                                   0.0, sbc[(0, g)][:],
                                                   op0=OP.max, op1=OP.mult)
                    rq[(g, ti)] = r
            for g in range(2):
                for ti in range(4):
                    ks = kvp.tile([128, TS], BF16, tag="ksc")
                    nc.vector.tensor_tensor(ks[:], qk_sb[(1, g, ti)][:],
                                            sbc[(1, g)][:], OP.mult)
                    sg = kvp.tile([128, TS], F32, tag="sig")
                    nc.scalar.activation(sg[:], ks[:], AF.Sigmoid)
                    sig[(g, ti)] = sg

            # ---- v / skv / scan / y / cproj1 ----
            for g in range(2):
                for ti in range(4):
                    m = g * 4 + ti
                    vp = mmps.tile([128, TS], F32, tag="mm")
                    for kt in range(2):
                        mm(vp[:], wq_ap(2, m, kt),
                           hT[:, kt * TS:(kt + 1) * TS], kt == 0, kt == 1)
                    skv = sig[(g, ti)]
                    nc.vector.tensor_tensor(skv[:], skv[:], vp[:], OP.mult)
                    nc.vector.tensor_tensor_scan(
                        skv[:], skv[:], skv[:], state[:, m:m + 1],
                        op0=OP.add, op1=OP.bypass)
                    nc.vector.tensor_copy(state[:, m:m + 1], skv[:, TS - 1:TS])
                    y = rq[(g, ti)]
                    nc.gpsimd.tensor_tensor(y[:], y[:], skv[:], OP.mult)
                    for rt in range(2):
                        mm(pp[rt][:],
                           w1c_sb[:, m * 256 + rt * 128:m * 256 + (rt + 1) * 128],
                           y[:], start=(m == 0), stop=(m == 7))

            for rt in range(2):
                p_sb = pbp.tile([128, TS], BF16, tag="pev", name="p_sb")
                nc.scalar.copy(p_sb[:], pp[rt][:])
                nc.sync.dma_start(p_in[s % ns2][s // ns2, rt, :, :], p_sb[:])

            # chunk j = s - ns2 of p is now complete on both halves: fire its
            # reduce-scatter immediately (it only occupies the collective
            # cores; Pool SEQ is released during the transfer)
            if s >= ns2:
                j = s - ns2
                nc.gpsimd.collective_compute(
                    "ReduceScatter", OP.add,
                    replica_groups=[[0, 1], [2, 3], [4, 5], [6, 7]],
                    ins=[p_in[j].opt()], outs=[p_out[j].opt()])

        for j in range(ns2):
            _emit_output_chunk(j)

            # chunk j of p is complete once slices j and ns2+j have landed;
            # emit its reduce-scatter + output projection right away so the
            # collective and the tail work overlap the remaining slices


# --------------------------------------------------------------------------
# host wrapper
# --------------------------------------------------------------------------

_NC_CACHE = {}


def _get_nc(t_len, ts):
    if (t_len, ts) not in _NC_CACHE:
        nc = _build(t_len, ts)
        nc.finalize()
        _NC_CACHE[(t_len, ts)] = nc
    return _NC_CACHE[(t_len, ts)]


def _run(x, qkv_w1, qkv_w2, cproj_w1, cproj_w2, t_len, ts=TS_DEFAULT, **spmd_kwargs):
    x = np.asarray(x, np.float32)
    w1 = np.asarray(qkv_w1, np.float32)
    w2 = np.asarray(qkv_w2, np.float32)
    cw1 = np.asarray(cproj_w1, np.float32)
    cw2 = np.asarray(cproj_w2, np.float32)
    nb = x.shape[0]

    import ml_dtypes
    c4, s4 = _tables(t_len)
    sel4 = np.ascontiguousarray(_sel4().astype(ml_dtypes.bfloat16))
    bs4 = np.ascontiguousarray(_sel4().T.astype(ml_dtypes.bfloat16))

    in_maps = []
    for ci in range(NCORES):
        b, hg = ci // 2, ci % 2
        d = _core_arrays(x[b % nb], w1, w2, cw1, cw2, hg, t_len)
        d.update(c4=c4, s4=s4, sel4=sel4, bs4=bs4)
        in_maps.append(d)

    nc = _get_nc(t_len, ts)
    res = bass_utils.run_bass_kernel_spmd(nc, in_maps, list(range(NCORES)),
                                          **spmd_kwargs)
    th = t_len // 2
    out = np.zeros((nb, t_len, C), np.float32)
    for ci in range(NCORES):
        b, hg = ci // 2, ci % 2
        if b < nb:
            out[b, hg * th:(hg + 1) * th, :] = res.results[ci]["out"]
    return out, res


def kernel(x, qkv_w1, qkv_w2, cproj_w1, cproj_w2):
    out, _ = _run(x, qkv_w1, qkv_w2, cproj_w1, cproj_w2, T_FULL)
    return out
